# revision 64
# baseline (speedup 1.0000x reference)
"""Distributed multi-head attention kernel for one TRN2 chip (8 NeuronCores).

Problem: b=2, n=2048, dim=1024, heads=16, hd=64.
  qkv = x @ Wqkv.T  (qkv-major split) -> RoPE(q,k) -> softmax(q k^T/8) v
  -> merge heads -> @ Wproj.T + bproj

Sharding: each core owns 2 heads (of 16) for BOTH batches. QKV projection,
RoPE and attention are fully head-local. Four 8-way AllToAlls (one per
batch x token-half, 128-token slices per core) redistribute attention
outputs head-major -> token-major as soon as each half's two q-blocks are
done; the first three hide under attention. ALL projection is deferred to
the tail: the attention phase is PE-bound while the final ~20us A2A
(8-rank ncfw latency is ~16us fixed) leaves the PE idle, so the three
already-delivered halves project inside that window. Core c outputs
tokens {1024h+128c : +128} for h in {0,1} of each batch; host reassembles.

Per-core inputs (see make_in_maps) are pre-transposed/pre-cast on the host so
no DMA-xbar transposes are needed (Tile serializes those globally):
  x        [1024, 4096] bf16  x^T: channels x flat tokens
  wqkv     [1024, 384]  bf16  (q|k|v rows for my heads)^T
  wproj    [1024, 1024] bf16  Wproj^T: [d', f]
  bproj    [1, 1024]    f32
  sin/cos  [2048, 64]   f32
  ident    [128, 128]   bf16  identity for PE transposes
  out      [512, 1024]  f32   rows = [b0h0, b0h1, b1h0, b1h1] x 128 tokens

All matmuls bf16 (PSUM accumulates f32). scoresT layout [k_j, q_i] (k
stationary, both heads row-packed across the 128 partitions) so softmax needs
no transposes: denominators come from a ones-column appended to v. exp on
ScalarE with fused 1/8 scale; no max subtraction (scores std ~2). The QKV
chain for each batch is software-pipelined into that batch's first
attention q-block (lag 4 tiles) so PE/ACT streams interleave; in qq0 the
qkc PSUM->SBUF copy rides the half-idle exp stream on ScalarE while the
rest stay on DVE, so neither engine gates the pipeline. b1's last 6 QKV
tiles emit inside b1-qq0, whose exp-bound phase has PE slack. x streams
in 512-token-block descriptors in consumption order; startup-critical
weight/sincos/x descriptors are split across the gpsimd/scalar/sync
queues (descriptor GEN is ~0.6us each and serializes per queue).
"""

import os
import numpy as np

NUM_CORES = 8
B, N, DIM, NH, HD = 2, 2048, 1024, 16, 64
T = B * N                 # 4096 flat tokens
HPC = NH // NUM_CORES     # 2 heads per core
P = 128
CT = DIM // P             # 8 channel tiles
SL = N // NUM_CORES       # 256 output tokens per core per batch
QW = HPC * HD             # 128
FQKV = 3 * QW             # 384
QB = 512                  # attention q-block width
TTH = N // P              # 16 token tiles per batch

_CACHE = {}


def _build_nc():
    from concourse import bacc, mybir, tile

    f32 = mybir.dt.float32
    bf16 = mybir.dt.bfloat16
    Exp = mybir.ActivationFunctionType.Exp
    mult = mybir.AluOpType.mult
    add = mybir.AluOpType.add

    nc = bacc.Bacc("TRN2", target_bir_lowering=False, debug=False,
                   num_devices=NUM_CORES)

    x_d = nc.dram_tensor("x", [DIM, T], bf16, kind="ExternalInput")
    wqkv_d = nc.dram_tensor("wqkv", [DIM, FQKV], bf16, kind="ExternalInput")
    wproj_d = nc.dram_tensor("wproj", [DIM, DIM], bf16, kind="ExternalInput")
    bproj_d = nc.dram_tensor("bproj", [1, DIM], f32, kind="ExternalInput")
    # cos and the rotate-half-negated sin, pre-rearranged on host to
    # [token-in-tile, tile, d] bf16 -- kills the f32 load + DVE negate prep
    # that used to gate the first RoPE by ~6us
    cosr_d = nc.dram_tensor("cosr", [P, 16 * HD], bf16, kind="ExternalInput")
    snegr_d = nc.dram_tensor("snegr", [P, 16 * HD], bf16, kind="ExternalInput")
    ident_d = nc.dram_tensor("ident", [P, P], bf16, kind="ExternalInput")
    out_d = nc.dram_tensor("out", [2 * SL, DIM], f32, kind="ExternalOutput")
    a2a_in = [[nc.dram_tensor(f"a2a_in{b}_{h}", [NUM_CORES * P, P], bf16)
               for h in range(2)] for b in range(B)]
    a2a_out = [[nc.dram_tensor(f"a2a_out{b}_{h}", [NUM_CORES * P, P], bf16)
                for h in range(2)] for b in range(B)]

    with tile.TileContext(nc) as tc:
        with (
            tc.tile_pool(name="persist", bufs=1) as pers,
            tc.tile_pool(name="work", bufs=4) as wp,
            tc.tile_pool(name="expp", bufs=6) as ep,
            tc.tile_pool(name="psA", bufs=2, space="PSUM") as psA,   # qkv/bc/proj/tp
            tc.tile_pool(name="psS", bufs=2, space="PSUM") as psS,   # scores
            tc.tile_pool(name="psV", bufs=1, space="PSUM") as psV,   # av accum
        ):
            # ---------------- persistent SBUF ----------------
            wqkvT = pers.tile([P, CT * FQKV], bf16)     # ct-block: [128c, 384f]
            wprojT = pers.tile([P, CT * DIM], bf16)     # dt-block: [128d', 1024f]
            xT = pers.tile([P, CT * T], bf16)           # ct-block: [128c, 4096t]
            qT = pers.tile([P, T], bf16)                # [d(2 heads), flat t]
            kT = pers.tile([P, T], bf16)
            v_sb = pers.tile([P, HPC * (T // P) * 65], bf16)
            aoT = pers.tile([P, T], bf16)               # [d', flat t]
            aoTr = pers.tile([P, B * NUM_CORES * SL], bf16)  # per b: [d'chnk, 256t]
            cos4 = pers.tile([P, 16 * 4 * HD], bf16)
            sneg4 = pers.tile([P, 16 * 4 * HD], bf16)
            ones_col = pers.tile([1, P], bf16)
            bias_bf = pers.tile([1, DIM], bf16)
            ident = pers.tile([P, P], bf16)

            nc.vector.memset(ones_col, 1.0)
            vv_ones = v_sb.rearrange("p (h t e) -> p h t e", h=HPC, t=T // P)
            nc.vector.memset(vv_ones[:, :, :, 64:65], 1.0)

            # ---------------- prep loads (no xbar transposes) ----------------
            # gpsimd queue: sincos (gate the first RoPE), wqkv (gates the
            # first matmul), ident, bias; sync queue: xT, first token-quarter
            # in half-column passes so tile-0's QKV matmuls start early
            cos_f = wp.tile([P, 16 * HD], bf16, tag="scload2", bufs=1)
            sneg_f = wp.tile([P, 16 * HD], bf16, tag="scload3", bufs=1)
            # descriptor GEN is ~0.6us per dma_start per queue and a single
            # descriptor's transfer runs ~20-60GB/s; split the startup-critical
            # loads across gpsimd + scalar (both free early) and put wqkv ct0
            # (which gates the first matmul) first, halved across 2 engines
            nc.gpsimd.dma_start(wqkvT[:, 0:FQKV // 2], wqkv_d[0:P, 0:FQKV // 2])
            nc.gpsimd.dma_start(wqkvT[:, FQKV // 2:FQKV],
                                wqkv_d[0:P, FQKV // 2:FQKV])
            nc.gpsimd.dma_start(cos_f, cosr_d.ap())
            nc.gpsimd.dma_start(sneg_f, snegr_d.ap())
            for ct in range(1, CT):
                q = nc.gpsimd if ct < 4 else nc.scalar
                q.dma_start(wqkvT[:, FQKV * ct:FQKV * (ct + 1)],
                            wqkv_d[P * ct:P * (ct + 1), :])
            nc.gpsimd.dma_start(ident, ident_d.ap())
            for half in range(2):
                for ct in range(CT):
                    q = nc.scalar if (half == 0 and ct >= 4) else nc.sync
                    q.dma_start(
                        xT[:, T * ct + 512 * half:T * ct + 512 * (half + 1)],
                        x_d[P * ct:P * (ct + 1), 512 * half:512 * (half + 1)])
            xTv = xT.rearrange("p (ct t) -> p ct t", ct=CT)
            xdv = x_d.ap().rearrange("(ct p) t -> p ct t", p=P)
            for blk in range(2, T // 512):   # b0 blocks 2-3, then b1 blocks
                nc.sync.dma_start(xTv[:, :, 512 * blk:512 * (blk + 1)],
                                  xdv[:, :, 512 * blk:512 * (blk + 1)])

            bt = wp.tile([1, DIM], f32, tag="bload", bufs=1)
            nc.gpsimd.dma_start(bt, bproj_d[:, :])
            nc.vector.tensor_copy(bias_bf, bt)
            c4 = cos4.rearrange("p (pt c d) -> p pt c d", pt=16, c=4)
            n4 = sneg4.rearrange("p (pt c d) -> p pt c d", pt=16, c=4)
            cf = cos_f.rearrange("p (pt d) -> p pt d", pt=16)
            nf = sneg_f.rearrange("p (pt d) -> p pt d", pt=16)
            for c in range(4):
                nc.vector.tensor_copy(c4[:, :, c, :], cf)
                nc.vector.tensor_copy(n4[:, :, c, :], nf)

            def emit_qkv_tile(b, tt, act_copies=False):
                """QKV matmul + RoPE + PE transposes for one 128-token tile.

                act_copies: route PSUM->SBUF copies to ScalarE (only safe in
                windows where the exp stream has slack, i.e. b0's first
                q-block)."""
                # act_copies: balanced split -- qkc + one tp copy ride the
                # half-idle qq0 exp stream on ScalarE, the rest stay on DVE
                cp = nc.scalar.copy if act_copies else nc.vector.tensor_copy
                ftt = TTH * b + tt
                qkvp = psA.tile([P, 512], f32, tag="mm", name="qkvp")
                for ct in range(CT):
                    base = T * ct + N * b
                    nc.tensor.matmul(
                        qkvp[:, 0:FQKV],
                        xT[:, base + P * tt:base + P * (tt + 1)],
                        wqkvT[:, FQKV * ct:FQKV * (ct + 1)],
                        start=(ct == 0), stop=(ct == CT - 1))
                qkc = wp.tile([P, 2 * QW], bf16, tag="qkc")
                cp(qkc, qkvp[:, 0:2 * QW])
                pt = tt % 16
                qk3 = qkc.rearrange("p (c d) -> p c d", c=4)
                t1 = wp.tile([P, 2 * QW], bf16, tag="t1")
                t13 = t1.rearrange("p (c d) -> p c d", c=4)
                nc.vector.tensor_tensor(t13[:, :, 0:32], qk3[:, :, 32:64],
                                        n4[:, pt, :, 0:32], mult)
                nc.vector.tensor_tensor(t13[:, :, 32:64], qk3[:, :, 0:32],
                                        n4[:, pt, :, 32:64], mult)
                qkcos = wp.tile([P, 2 * QW], bf16, tag="qkcos")
                nc.vector.tensor_tensor(
                    qkcos, qkc, cos4[:, 4 * HD * pt:4 * HD * (pt + 1)], mult)
                qrope = wp.tile([P, QW], bf16, tag="qrope")
                krope = wp.tile([P, QW], bf16, tag="krope")
                nc.vector.tensor_tensor(qrope, qkcos[:, 0:QW], t1[:, 0:QW], add)
                nc.vector.tensor_tensor(krope, qkcos[:, QW:2 * QW],
                                        t1[:, QW:2 * QW], add)
                # in qq0 the scores slots are mostly idle; borrowing them
                # for the transpose tile lets qkvp double-buffer on psA
                tpool, ttag = (psS, "scores") if act_copies else (psA, "mm")
                tp = tpool.tile([P, 2 * P], bf16, tag=ttag, name="tp")
                nc.tensor.transpose(tp[:, 0:P], qrope, ident)
                nc.tensor.transpose(tp[:, P:2 * P], krope, ident)
                nc.vector.tensor_copy(qT[:, P * ftt:P * (ftt + 1)], tp[:, 0:P])
                nc.vector.tensor_copy(kT[:, P * ftt:P * (ftt + 1)],
                                      tp[:, P:2 * P])
                vv = v_sb.rearrange("p (h t e) -> p h t e", h=HPC, t=T // P)
                nc.vector.tensor_copy(
                    vv[:, :, ftt, 0:HD],
                    qkvp[:, 2 * QW:3 * QW].rearrange("p (h d) -> p h d", h=HPC))

            def emit_scores_exp(b, qq, jt):
                """Scores + exp for one (q-block, j-tile); returns the exp tile."""
                ftt = TTH * b + jt
                sp = psS.tile([P, HPC * QB], f32, tag="scores", name="sp")
                for h in range(HPC):
                    nc.tensor.matmul(
                        sp[:, QB * h:QB * (h + 1)],
                        kT[HD * h:HD * (h + 1), P * ftt:P * (ftt + 1)],
                        qT[HD * h:HD * (h + 1),
                           N * b + QB * qq:N * b + QB * (qq + 1)],
                        start=True, stop=True)
                et = ep.tile([P, HPC * QB], bf16, tag="expT", name="et")
                nc.scalar.activation(et, sp, Exp, scale=float(HD) ** -0.5)
                return et

            def emit_av(b, qq, jt, av, et):
                ftt = TTH * b + jt
                for h in range(HPC):
                    blk = (h * (T // P) + ftt) * 65
                    nc.tensor.matmul(av[h], v_sb[:, blk:blk + 65],
                                     et[:, QB * h:QB * (h + 1)],
                                     start=(jt == 0), stop=(jt == TTH - 1))

            def emit_avf(avp):
                avf = wp.tile([65, HPC * QB], f32, tag="avf", bufs=2, name="avf")
                nc.vector.tensor_copy(avf, avp)
                return avf

            def emit_norm_head(b, qq, avf, h):
                """Denominator broadcast + reciprocal + normalize, one head."""
                sums = wp.tile([1, QB], bf16, tag="sums", name="sums")
                nc.vector.tensor_copy(sums, avf[64:65, QB * h:QB * (h + 1)])
                bc = psA.tile([64, QB], f32, tag="mm", name="bc")
                nc.tensor.matmul(bc, ones_col[:, 0:64], sums,
                                 start=True, stop=True)
                rc = wp.tile([64, QB], f32, tag="recip", bufs=2, name="rc")
                nc.vector.reciprocal_approx_fast(rc, bc)
                nc.vector.tensor_tensor(
                    aoT[HD * h:HD * (h + 1),
                        N * b + QB * qq:N * b + QB * (qq + 1)],
                    avf[0:64, QB * h:QB * (h + 1)], rc, mult)

            def emit_stage(b, qq):
                """A2A staging of this q-block's two 256-token chunks."""
                # stage sub-chunks 4*(qq%2)..+3 of a2a_in[b][qq//2]
                r0 = 4 * P * (qq % 2)
                a2i = a2a_in[b][qq // 2][r0:r0 + 4 * P].rearrange(
                    "(c p) t -> p c t", p=P)
                nc.sync.dma_start(
                    a2i, aoT[:, N * b + QB * qq:N * b + QB * (qq + 1)].rearrange(
                        "p (c t) -> p c t", c=4))

            def emit_a2a(b, h):
                nc.gpsimd.collective_compute(
                    "AllToAll", mybir.AluOpType.bypass,
                    replica_groups=[list(range(NUM_CORES))],
                    ins=[a2a_in[b][h].ap().opt()],
                    outs=[a2a_out[b][h].ap().opt()])

            def proj_pieces(b, h, split=False):
                """Fetch A2A(b, h) result and project my 128-token slice.
                Returns emission pieces (fetch, fb0, fb1) for interleaving.
                split=True halves the fetch and stores across two DMA queues
                (used for the last piece, which sits on the tail critical
                path after the final A2A)."""
                rbase = NUM_CORES * P * (2 * b + h)

                def fetch():
                    a2o = a2a_out[b][h].ap().rearrange("(c p) t -> p c t", p=P)
                    dst = aoTr[:, rbase:rbase + NUM_CORES * P].rearrange(
                        "p (c t) -> p c t", c=NUM_CORES)
                    if split:
                        nc.sync.dma_start(dst[:, 0:4], a2o[:, 0:4])
                        nc.scalar.dma_start(dst[:, 4:8], a2o[:, 4:8])
                    else:
                        nc.sync.dma_start(dst, a2o)

                def fb_piece(fb):
                    def run():
                        pp = psA.tile([P, 512], f32, tag="mm", name="proj")
                        for dt in range(CT):
                            lo = rbase + P * dt
                            nc.tensor.matmul(
                                pp, aoTr[:, lo:lo + P],
                                wprojT[:, DIM * dt + 512 * fb:DIM * dt + 512 * (fb + 1)],
                                start=(dt == 0), stop=False)
                        nc.tensor.matmul(pp, ones_col[:, 0:P],
                                         bias_bf[:, 512 * fb:512 * (fb + 1)],
                                         start=False, stop=True)
                        ob = wp.tile([P, 512], f32, tag="ob", bufs=2, name="ob")
                        nc.scalar.copy(ob, pp)
                        orow = 2 * P * b + P * h
                        od = out_d[orow:orow + P, 512 * fb:512 * (fb + 1)]
                        if split:
                            nc.sync.dma_start(od[0:64, :], ob[0:64, :])
                            nc.scalar.dma_start(od[64:P, :], ob[64:P, :])
                        else:
                            nc.sync.dma_start(od, ob)
                    return run
                return [fetch, fb_piece(0), fb_piece(1)]



            # ---------------- main schedule ----------------
            # b0: qq0 software-pipelines b0's QKV chain (ScalarE-assisted
            # copies — exp has slack there); qq1-3 interleave b1's QKV tiles
            # (DVE copies). AV matmuls trail scores/exp by one iteration and
            # the last AV + avf copy of each q-block is carried into the next
            # block's first iteration, so the boundary never stalls the exp
            # stream. Batch-0's A2A + projection hide under b1's attention.
            LAG = 4
            state = {"pend": None, "carry": None}

            def flush_carry():
                if state["carry"] is not None:
                    state["carry"]()
                    state["carry"] = None

            def set_carry(b, qq, avp, av, et):
                def fn():
                    emit_av(b, qq, TTH - 1, av, et)
                    state["pend"] = (b, qq, emit_avf(avp), 0)
                state["carry"] = fn

            def flush_pend_step(upto):
                """Emit the pending norm up to head `upto`; upto==2 also
                stages. Spreading the two heads' broadcast matmuls across
                jt slots avoids bunching PE work at one qq-boundary slot."""
                if state["pend"] is None:
                    return
                b, qq, avf, nh = state["pend"]
                while nh < min(upto, HPC):
                    emit_norm_head(b, qq, avf, nh)
                    nh += 1
                if upto >= 2:
                    emit_stage(b, qq)
                    state["pend"] = None
                else:
                    state["pend"] = (b, qq, avf, nh)

            def flush_pend():
                flush_pend_step(2)

            for qq in range(N // QB):
                avp = psV.tile([65, HPC * QB], f32, tag="av", name="avp")
                av = [avp[:, QB * h:QB * (h + 1)] for h in range(HPC)]
                prev_et = None
                if qq == 0:
                    for step in range(TTH + LAG):
                        if step < TTH:
                            emit_qkv_tile(0, step, act_copies=True)
                        if step >= LAG:
                            jt = step - LAG
                            et = emit_scores_exp(0, qq, jt)
                            if jt >= 1:
                                emit_av(0, qq, jt - 1, av, prev_et)
                            prev_et = et
                else:
                    for jt in range(TTH):
                        et = emit_scores_exp(0, qq, jt)
                        if jt == 0:
                            flush_carry()
                        else:
                            emit_av(0, qq, jt - 1, av, prev_et)
                        prev_et = et
                        if jt == 2:
                            flush_pend_step(1)
                        elif jt == 4:
                            flush_pend()
                            if qq == 2:
                                emit_a2a(0, 0)   # b0 qq0+qq1 staged
                        if jt % 3 == 0 and jt <= 9:
                            nb1 = 4 * (qq - 1) + jt // 3
                            if nb1 < 10:
                                emit_qkv_tile(1, nb1)
                set_carry(0, qq, avp, av, prev_et)
                if qq == 1:
                    nc.sync.dma_start(
                        wprojT.rearrange("p (dt f) -> p dt f", dt=CT),
                        wproj_d.ap().rearrange("(dt p) f -> p dt f", p=P))
            for qq in range(N // QB):
                avp = psV.tile([65, HPC * QB], f32, tag="av", name="avp")
                av = [avp[:, QB * h:QB * (h + 1)] for h in range(HPC)]
                prev_et = None
                for jt in range(TTH):
                    et = emit_scores_exp(1, qq, jt)
                    if jt == 0:
                        flush_carry()
                    else:
                        emit_av(1, qq, jt - 1, av, prev_et)
                    prev_et = et
                    if jt == 2:
                        flush_pend_step(1)
                    elif jt == 4:
                        flush_pend()
                        if qq == 0:
                            emit_a2a(0, 1)   # b0 qq2+qq3 staged
                        elif qq == 2:
                            emit_a2a(1, 0)   # b1 qq0+qq1 staged

                    if qq == 0 and jt % 2 == 1 and jt <= 11:
                        # b1 tiles 10-15: this phase is exp-bound, the PE
                        # has slack; tile 10+j/2 lands >=2 jts before use
                        emit_qkv_tile(1, 10 + jt // 2)
                if qq < 3:
                    set_carry(1, qq, avp, av, prev_et)
                else:
                    final = (avp, av, prev_et)
            # final q-block: everything below is on the tail critical path
            # (last exp -> A2A trigger), so skip the avf PSUM->SBUF copy
            # (psV is never reused) and split the staging across two queues.
            emit_av(1, 3, TTH - 1, final[1], final[2])
            avp = final[0]
            for h in range(HPC):
                sums = wp.tile([1, QB], bf16, tag="sums", name="sums")
                nc.vector.tensor_copy(sums, avp[64:65, QB * h:QB * (h + 1)])
                bc = psA.tile([64, QB], f32, tag="mm", name="bc")
                nc.tensor.matmul(bc, ones_col[:, 0:64], sums,
                                 start=True, stop=True)
                rc = wp.tile([64, QB], f32, tag="recip", bufs=2, name="rc")
                nc.vector.reciprocal_approx_fast(rc, bc)
                nc.vector.tensor_tensor(
                    aoT[HD * h:HD * (h + 1), N + QB * 3:N + QB * 4],
                    avp[0:64, QB * h:QB * (h + 1)], rc, mult)
            aos = aoT[:, N + QB * 3:N + QB * 4].rearrange(
                "p (c t) -> p c t", c=4)
            for k in range(2):
                a2i = a2a_in[1][1][(4 + 2 * k) * P:(6 + 2 * k) * P].rearrange(
                    "(c p) t -> p c t", p=P)
                q = nc.sync if k == 0 else nc.scalar
                q.dma_start(a2i, aos[:, 2 * k:2 * (k + 1)])
            emit_a2a(1, 1)
            # all projection runs in the tail: the attention phase is
            # PE-bound (interleaving proj there costs span 1:1) while the
            # final A2A leaves the PE idle -- overlap it with the three
            # already-delivered halves, then the last one.
            for (pb, ph) in [(0, 0), (0, 1), (1, 0), (1, 1)]:
                for piece in proj_pieces(pb, ph, split=(pb == 1 and ph == 1)):
                    piece()

    nc.compile()
    return nc


def _get_nc():
    if "nc" not in _CACHE:
        _CACHE["nc"] = _build_nc()
    return _CACHE["nc"]


def make_in_maps(x, Wqkv, Wproj, bproj, sin, cos):
    """Shard full (f32) inputs into per-core in_maps (pre-cast + pre-transposed)."""
    import ml_dtypes
    bf16 = ml_dtypes.bfloat16
    xT = np.ascontiguousarray(
        np.asarray(x, np.float32).reshape(T, DIM).astype(bf16).T)
    Wqkv = np.asarray(Wqkv, np.float32).astype(bf16)
    WprojT = np.ascontiguousarray(np.asarray(Wproj, np.float32).astype(bf16).T)
    bproj = np.asarray(bproj, np.float32).reshape(1, DIM)
    sin = np.asarray(sin, np.float32)
    cos = np.asarray(cos, np.float32)
    # [n, d] -> [token-in-tile, tile*d] bf16; sneg = rotate-half sign pattern
    sneg = np.concatenate([-sin[:, :HD // 2], sin[:, HD // 2:]], axis=1)
    cosr = np.ascontiguousarray(
        cos.reshape(16, P, HD).transpose(1, 0, 2).reshape(P, 16 * HD)
    ).astype(bf16)
    snegr = np.ascontiguousarray(
        sneg.reshape(16, P, HD).transpose(1, 0, 2).reshape(P, 16 * HD)
    ).astype(bf16)
    ident = np.eye(P, dtype=bf16)
    in_maps = []
    for c in range(NUM_CORES):
        r = P * c
        wq = Wqkv[r:r + P]
        wk = Wqkv[DIM + r:DIM + r + P]
        wv = Wqkv[2 * DIM + r:2 * DIM + r + P]
        in_maps.append({
            "x": xT,
            "wqkv": np.ascontiguousarray(np.concatenate([wq, wk, wv], 0).T),
            "wproj": WprojT,
            "bproj": bproj,
            "cosr": cosr,
            "snegr": snegr,
            "ident": ident,
        })
    return in_maps


def reassemble(outs):
    """outs[c] = [512, 1024] f32, rows [b0h0|b0h1|b1h0|b1h1] x 128 t."""
    out = np.empty((B, N, DIM), np.float32)
    for c in range(NUM_CORES):
        o = outs[c]
        for b in range(B):
            for h in range(2):
                t0 = N // 2 * h + P * c
                out[b, t0:t0 + P] = o[2 * P * b + P * h:2 * P * b + P * (h + 1)]
    return out


def kernel(x, Wqkv, Wproj, bproj, sin, cos):
    from concourse.bass_utils import run_bass_kernel_spmd

    nc = _get_nc()
    in_maps = make_in_maps(x, Wqkv, Wproj, bproj, sin, cos)
    trace = bool(int(os.environ.get("KERNEL_TRACE", "0")))
    res = run_bass_kernel_spmd(nc, in_maps, core_ids=list(range(NUM_CORES)),
                               trace=trace)
    _CACHE["last_result"] = res
    return reassemble([res.results[c]["out"] for c in range(NUM_CORES)])



# revision 65
# speedup vs baseline: 1.0085x; 1.0085x over previous
"""Distributed multi-head attention kernel for one TRN2 chip (8 NeuronCores).

Problem: b=2, n=2048, dim=1024, heads=16, hd=64.
  qkv = x @ Wqkv.T  (qkv-major split) -> RoPE(q,k) -> softmax(q k^T/8) v
  -> merge heads -> @ Wproj.T + bproj

Sharding: each core owns 2 heads (of 16) for BOTH batches. QKV projection,
RoPE and attention are fully head-local. Four 8-way AllToAlls (one per
batch x token-half, 128-token slices per core) redistribute attention
outputs head-major -> token-major as soon as each half's two q-blocks are
done; the first three hide under attention. ALL projection is deferred to
the tail: the attention phase is PE-bound while the final ~20us A2A
(8-rank ncfw latency is ~16us fixed) leaves the PE idle, so the three
already-delivered halves project inside that window. Core c outputs
tokens {1024h+128c : +128} for h in {0,1} of each batch; host reassembles.

Per-core inputs (see make_in_maps) are pre-transposed/pre-cast on the host so
no DMA-xbar transposes are needed (Tile serializes those globally):
  x        [1024, 4096] bf16  x^T: channels x flat tokens
  wqkv     [1024, 384]  bf16  (q|k|v rows for my heads)^T
  wproj    [1024, 1024] bf16  Wproj^T: [d', f]
  bproj    [1, 1024]    f32
  sin/cos  [2048, 64]   f32
  ident    [128, 128]   bf16  identity for PE transposes
  out      [512, 1024]  f32   rows = [b0h0, b0h1, b1h0, b1h1] x 128 tokens

All matmuls bf16 (PSUM accumulates f32). scoresT layout [k_j, q_i] (k
stationary, both heads row-packed across the 128 partitions) so softmax needs
no transposes: denominators come from a ones-column appended to v. exp on
ScalarE with fused 1/8 scale; no max subtraction (scores std ~2). The QKV
chain for each batch is software-pipelined into that batch's first
attention q-block (lag 4 tiles) so PE/ACT streams interleave; in qq0 the
qkc PSUM->SBUF copy rides the half-idle exp stream on ScalarE while the
rest stay on DVE, so neither engine gates the pipeline. b1's last 6 QKV
tiles emit inside b1-qq0, whose exp-bound phase has PE slack. x streams
in 512-token-block descriptors in consumption order; startup-critical
weight/sincos/x descriptors are split across the gpsimd/scalar/sync
queues (descriptor GEN is ~0.6us each and serializes per queue).
"""

import os
import numpy as np

NUM_CORES = 8
B, N, DIM, NH, HD = 2, 2048, 1024, 16, 64
T = B * N                 # 4096 flat tokens
HPC = NH // NUM_CORES     # 2 heads per core
P = 128
CT = DIM // P             # 8 channel tiles
SL = N // NUM_CORES       # 256 output tokens per core per batch
QW = HPC * HD             # 128
FQKV = 3 * QW             # 384
QB = 512                  # attention q-block width
TTH = N // P              # 16 token tiles per batch

_CACHE = {}


def _build_nc():
    from concourse import bacc, mybir, tile

    f32 = mybir.dt.float32
    bf16 = mybir.dt.bfloat16
    Exp = mybir.ActivationFunctionType.Exp
    mult = mybir.AluOpType.mult
    add = mybir.AluOpType.add

    nc = bacc.Bacc("TRN2", target_bir_lowering=False, debug=False,
                   num_devices=NUM_CORES)

    x_d = nc.dram_tensor("x", [DIM, T], bf16, kind="ExternalInput")
    wqkv_d = nc.dram_tensor("wqkv", [DIM, FQKV], bf16, kind="ExternalInput")
    wproj_d = nc.dram_tensor("wproj", [DIM, DIM], bf16, kind="ExternalInput")
    bproj_d = nc.dram_tensor("bproj", [1, DIM], f32, kind="ExternalInput")
    # cos and the rotate-half-negated sin, pre-rearranged on host to
    # [token-in-tile, tile, d] bf16 -- kills the f32 load + DVE negate prep
    # that used to gate the first RoPE by ~6us
    cosr_d = nc.dram_tensor("cosr", [P, 16 * HD], bf16, kind="ExternalInput")
    snegr_d = nc.dram_tensor("snegr", [P, 16 * HD], bf16, kind="ExternalInput")
    ident_d = nc.dram_tensor("ident", [P, P], bf16, kind="ExternalInput")
    out_d = nc.dram_tensor("out", [2 * SL, DIM], f32, kind="ExternalOutput")
    a2a_in = [[nc.dram_tensor(f"a2a_in{b}_{h}", [NUM_CORES * P, P], bf16)
               for h in range(2)] for b in range(B)]
    a2a_out = [[nc.dram_tensor(f"a2a_out{b}_{h}", [NUM_CORES * P, P], bf16)
                for h in range(2)] for b in range(B)]

    with tile.TileContext(nc) as tc:
        with (
            tc.tile_pool(name="persist", bufs=1) as pers,
            tc.tile_pool(name="work", bufs=4) as wp,
            tc.tile_pool(name="expp", bufs=6) as ep,
            tc.tile_pool(name="psA", bufs=2, space="PSUM") as psA,   # qkv/bc/proj/tp
            tc.tile_pool(name="psS", bufs=2, space="PSUM") as psS,   # scores
            tc.tile_pool(name="psV", bufs=1, space="PSUM") as psV,   # av accum
        ):
            # ---------------- persistent SBUF ----------------
            wqkvT = pers.tile([P, CT * FQKV], bf16)     # ct-block: [128c, 384f]
            wprojT = pers.tile([P, CT * DIM], bf16)     # dt-block: [128d', 1024f]
            xT = pers.tile([P, CT * T], bf16)           # ct-block: [128c, 4096t]
            qT = pers.tile([P, T], bf16)                # [d(2 heads), flat t]
            kT = pers.tile([P, T], bf16)
            v_sb = pers.tile([P, HPC * (T // P) * 65], bf16)
            aoT = pers.tile([P, T], bf16)               # [d', flat t]
            aoTr = pers.tile([P, B * NUM_CORES * SL], bf16)  # per b: [d'chnk, 256t]
            cos4 = pers.tile([P, 16 * 4 * HD], bf16)
            sneg4 = pers.tile([P, 16 * 4 * HD], bf16)
            ones_col = pers.tile([1, P], bf16)
            bias_bf = pers.tile([1, DIM], bf16)
            ident = pers.tile([P, P], bf16)

            nc.vector.memset(ones_col, 1.0)
            vv_ones = v_sb.rearrange("p (h t e) -> p h t e", h=HPC, t=T // P)
            nc.vector.memset(vv_ones[:, :, :, 64:65], 1.0)

            # ---------------- prep loads (no xbar transposes) ----------------
            # gpsimd queue: sincos (gate the first RoPE), wqkv (gates the
            # first matmul), ident, bias; sync queue: xT, first token-quarter
            # in half-column passes so tile-0's QKV matmuls start early
            cos_f = wp.tile([P, 16 * HD], bf16, tag="scload2", bufs=1)
            sneg_f = wp.tile([P, 16 * HD], bf16, tag="scload3", bufs=1)
            # descriptor GEN is ~0.6us per dma_start per queue and a single
            # descriptor's transfer runs ~20-60GB/s; split the startup-critical
            # loads across gpsimd + scalar (both free early) and put wqkv ct0
            # (which gates the first matmul) first, halved across 2 engines
            nc.gpsimd.dma_start(wqkvT[:, 0:FQKV // 2], wqkv_d[0:P, 0:FQKV // 2])
            nc.gpsimd.dma_start(wqkvT[:, FQKV // 2:FQKV],
                                wqkv_d[0:P, FQKV // 2:FQKV])
            nc.gpsimd.dma_start(cos_f, cosr_d.ap())
            nc.gpsimd.dma_start(sneg_f, snegr_d.ap())
            for ct in range(1, CT):
                q = nc.gpsimd if ct < 4 else nc.scalar
                q.dma_start(wqkvT[:, FQKV * ct:FQKV * (ct + 1)],
                            wqkv_d[P * ct:P * (ct + 1), :])
            nc.gpsimd.dma_start(ident, ident_d.ap())
            for half in range(2):
                for ct in range(CT):
                    q = nc.scalar if (half == 0 and ct >= 4) else nc.sync
                    q.dma_start(
                        xT[:, T * ct + 512 * half:T * ct + 512 * (half + 1)],
                        x_d[P * ct:P * (ct + 1), 512 * half:512 * (half + 1)])
            xTv = xT.rearrange("p (ct t) -> p ct t", ct=CT)
            xdv = x_d.ap().rearrange("(ct p) t -> p ct t", p=P)
            for blk in range(2, T // 512):   # b0 blocks 2-3, then b1 blocks
                nc.sync.dma_start(xTv[:, :, 512 * blk:512 * (blk + 1)],
                                  xdv[:, :, 512 * blk:512 * (blk + 1)])

            bt = wp.tile([1, DIM], f32, tag="bload", bufs=1)
            nc.gpsimd.dma_start(bt, bproj_d[:, :])
            nc.vector.tensor_copy(bias_bf, bt)
            c4 = cos4.rearrange("p (pt c d) -> p pt c d", pt=16, c=4)
            n4 = sneg4.rearrange("p (pt c d) -> p pt c d", pt=16, c=4)
            cf = cos_f.rearrange("p (pt d) -> p pt d", pt=16)
            nf = sneg_f.rearrange("p (pt d) -> p pt d", pt=16)
            for c in range(4):
                nc.vector.tensor_copy(c4[:, :, c, :], cf)
                nc.vector.tensor_copy(n4[:, :, c, :], nf)

            def emit_qkv_tile(b, tt, act_copies=False):
                """QKV matmul + RoPE + PE transposes for one 128-token tile.

                act_copies: route PSUM->SBUF copies to ScalarE (only safe in
                windows where the exp stream has slack, i.e. b0's first
                q-block)."""
                # act_copies: balanced split -- qkc + one tp copy ride the
                # half-idle qq0 exp stream on ScalarE, the rest stay on DVE
                cp = nc.scalar.copy if act_copies else nc.vector.tensor_copy
                ftt = TTH * b + tt
                qkvp = psA.tile([P, 512], f32, tag="mm", name="qkvp")
                for ct in range(CT):
                    base = T * ct + N * b
                    nc.tensor.matmul(
                        qkvp[:, 0:FQKV],
                        xT[:, base + P * tt:base + P * (tt + 1)],
                        wqkvT[:, FQKV * ct:FQKV * (ct + 1)],
                        start=(ct == 0), stop=(ct == CT - 1))
                qkc = wp.tile([P, 2 * QW], bf16, tag="qkc")
                cp(qkc, qkvp[:, 0:2 * QW])
                pt = tt % 16
                qk3 = qkc.rearrange("p (c d) -> p c d", c=4)
                t1 = wp.tile([P, 2 * QW], bf16, tag="t1")
                t13 = t1.rearrange("p (c d) -> p c d", c=4)
                nc.vector.tensor_tensor(t13[:, :, 0:32], qk3[:, :, 32:64],
                                        n4[:, pt, :, 0:32], mult)
                nc.vector.tensor_tensor(t13[:, :, 32:64], qk3[:, :, 0:32],
                                        n4[:, pt, :, 32:64], mult)
                qkcos = wp.tile([P, 2 * QW], bf16, tag="qkcos")
                nc.vector.tensor_tensor(
                    qkcos, qkc, cos4[:, 4 * HD * pt:4 * HD * (pt + 1)], mult)
                qrope = wp.tile([P, QW], bf16, tag="qrope")
                krope = wp.tile([P, QW], bf16, tag="krope")
                nc.vector.tensor_tensor(qrope, qkcos[:, 0:QW], t1[:, 0:QW], add)
                nc.vector.tensor_tensor(krope, qkcos[:, QW:2 * QW],
                                        t1[:, QW:2 * QW], add)
                # in qq0 the scores slots are mostly idle; borrowing them
                # for the transpose tile lets qkvp double-buffer on psA
                tpool, ttag = (psS, "scores") if act_copies else (psA, "mm")
                tp = tpool.tile([P, 2 * P], bf16, tag=ttag, name="tp")
                nc.tensor.transpose(tp[:, 0:P], qrope, ident)
                nc.tensor.transpose(tp[:, P:2 * P], krope, ident)
                nc.vector.tensor_copy(qT[:, P * ftt:P * (ftt + 1)], tp[:, 0:P])
                nc.vector.tensor_copy(kT[:, P * ftt:P * (ftt + 1)],
                                      tp[:, P:2 * P])
                vv = v_sb.rearrange("p (h t e) -> p h t e", h=HPC, t=T // P)
                nc.vector.tensor_copy(
                    vv[:, :, ftt, 0:HD],
                    qkvp[:, 2 * QW:3 * QW].rearrange("p (h d) -> p h d", h=HPC))

            def emit_scores_exp(b, qq, jt):
                """Scores + exp for one (q-block, j-tile); returns the exp tile."""
                ftt = TTH * b + jt
                sp = psS.tile([P, HPC * QB], f32, tag="scores", name="sp")
                for h in range(HPC):
                    nc.tensor.matmul(
                        sp[:, QB * h:QB * (h + 1)],
                        kT[HD * h:HD * (h + 1), P * ftt:P * (ftt + 1)],
                        qT[HD * h:HD * (h + 1),
                           N * b + QB * qq:N * b + QB * (qq + 1)],
                        start=True, stop=True)
                et = ep.tile([P, HPC * QB], bf16, tag="expT", name="et")
                nc.scalar.activation(et, sp, Exp, scale=float(HD) ** -0.5)
                return et

            def emit_av(b, qq, jt, av, et):
                ftt = TTH * b + jt
                for h in range(HPC):
                    blk = (h * (T // P) + ftt) * 65
                    nc.tensor.matmul(av[h], v_sb[:, blk:blk + 65],
                                     et[:, QB * h:QB * (h + 1)],
                                     start=(jt == 0), stop=(jt == TTH - 1))

            def emit_avf(avp):
                avf = wp.tile([65, HPC * QB], f32, tag="avf", bufs=2, name="avf")
                nc.vector.tensor_copy(avf, avp)
                return avf

            def emit_norm_head(b, qq, avf, h):
                """Denominator broadcast + reciprocal + normalize, one head."""
                sums = wp.tile([1, QB], bf16, tag="sums", name="sums")
                nc.vector.tensor_copy(sums, avf[64:65, QB * h:QB * (h + 1)])
                bc = psA.tile([64, QB], f32, tag="mm", name="bc")
                nc.tensor.matmul(bc, ones_col[:, 0:64], sums,
                                 start=True, stop=True)
                rc = wp.tile([64, QB], f32, tag="recip", bufs=2, name="rc")
                nc.vector.reciprocal_approx_fast(rc, bc)
                nc.vector.tensor_tensor(
                    aoT[HD * h:HD * (h + 1),
                        N * b + QB * qq:N * b + QB * (qq + 1)],
                    avf[0:64, QB * h:QB * (h + 1)], rc, mult)

            def emit_stage(b, qq):
                """A2A staging of this q-block's two 256-token chunks."""
                # stage sub-chunks 4*(qq%2)..+3 of a2a_in[b][qq//2]
                r0 = 4 * P * (qq % 2)
                a2i = a2a_in[b][qq // 2][r0:r0 + 4 * P].rearrange(
                    "(c p) t -> p c t", p=P)
                nc.sync.dma_start(
                    a2i, aoT[:, N * b + QB * qq:N * b + QB * (qq + 1)].rearrange(
                        "p (c t) -> p c t", c=4))

            def emit_a2a(b, h):
                nc.gpsimd.collective_compute(
                    "AllToAll", mybir.AluOpType.bypass,
                    replica_groups=[list(range(NUM_CORES))],
                    ins=[a2a_in[b][h].ap().opt()],
                    outs=[a2a_out[b][h].ap().opt()])

            def proj_pieces(b, h, split=False):
                """Fetch A2A(b, h) result and project my 128-token slice.
                Returns emission pieces (fetch, fb0, fb1) for interleaving.
                split=True halves the fetch and stores across two DMA queues
                (used for the last piece, which sits on the tail critical
                path after the final A2A)."""
                rbase = NUM_CORES * P * (2 * b + h)

                def fetch():
                    a2o = a2a_out[b][h].ap().rearrange("(c p) t -> p c t", p=P)
                    dst = aoTr[:, rbase:rbase + NUM_CORES * P].rearrange(
                        "p (c t) -> p c t", c=NUM_CORES)
                    if split:
                        nc.sync.dma_start(dst[:, 0:4], a2o[:, 0:4])
                        nc.scalar.dma_start(dst[:, 4:8], a2o[:, 4:8])
                    else:
                        nc.sync.dma_start(dst, a2o)

                def fb_piece(fb):
                    def run():
                        pp = psA.tile([P, 512], f32, tag="mm", name="proj")
                        for dt in range(CT):
                            lo = rbase + P * dt
                            nc.tensor.matmul(
                                pp, aoTr[:, lo:lo + P],
                                wprojT[:, DIM * dt + 512 * fb:DIM * dt + 512 * (fb + 1)],
                                start=(dt == 0), stop=False)
                        nc.tensor.matmul(pp, ones_col[:, 0:P],
                                         bias_bf[:, 512 * fb:512 * (fb + 1)],
                                         start=False, stop=True)
                        ob = wp.tile([P, 512], f32, tag="ob", bufs=2, name="ob")
                        nc.scalar.copy(ob, pp)
                        orow = 2 * P * b + P * h
                        od = out_d[orow:orow + P, 512 * fb:512 * (fb + 1)]
                        if split:
                            nc.sync.dma_start(od[0:64, :], ob[0:64, :])
                            nc.scalar.dma_start(od[64:P, :], ob[64:P, :])
                        else:
                            nc.sync.dma_start(od, ob)
                    return run
                return [fetch, fb_piece(0), fb_piece(1)]



            # ---------------- main schedule ----------------
            # b0: qq0 software-pipelines b0's QKV chain (ScalarE-assisted
            # copies — exp has slack there); qq1-3 interleave b1's QKV tiles
            # (DVE copies). AV matmuls trail scores/exp by one iteration and
            # the last AV + avf copy of each q-block is carried into the next
            # block's first iteration, so the boundary never stalls the exp
            # stream. Batch-0's A2A + projection hide under b1's attention.
            LAG = 4
            state = {"pend": None, "carry": None}

            def flush_carry():
                if state["carry"] is not None:
                    state["carry"]()
                    state["carry"] = None

            def set_carry(b, qq, avp, av, et):
                def fn():
                    emit_av(b, qq, TTH - 1, av, et)
                    state["pend"] = (b, qq, emit_avf(avp), 0)
                state["carry"] = fn

            def flush_pend_step(upto):
                """Emit the pending norm up to head `upto`; upto==2 also
                stages. Spreading the two heads' broadcast matmuls across
                jt slots avoids bunching PE work at one qq-boundary slot."""
                if state["pend"] is None:
                    return
                b, qq, avf, nh = state["pend"]
                while nh < min(upto, HPC):
                    emit_norm_head(b, qq, avf, nh)
                    nh += 1
                if upto >= 2:
                    emit_stage(b, qq)
                    state["pend"] = None
                else:
                    state["pend"] = (b, qq, avf, nh)

            def flush_pend():
                flush_pend_step(2)

            for qq in range(N // QB):
                avp = psV.tile([65, HPC * QB], f32, tag="av", name="avp")
                av = [avp[:, QB * h:QB * (h + 1)] for h in range(HPC)]
                prev_et = None
                if qq == 0:
                    for step in range(TTH + LAG):
                        if step < TTH:
                            emit_qkv_tile(0, step, act_copies=True)
                        if step >= LAG:
                            jt = step - LAG
                            et = emit_scores_exp(0, qq, jt)
                            if jt >= 1:
                                emit_av(0, qq, jt - 1, av, prev_et)
                            prev_et = et
                else:
                    for jt in range(TTH):
                        et = emit_scores_exp(0, qq, jt)
                        if jt == 0:
                            flush_carry()
                        else:
                            emit_av(0, qq, jt - 1, av, prev_et)
                        prev_et = et
                        if jt == 2:
                            flush_pend_step(1)
                        elif jt == 4:
                            flush_pend()
                            if qq == 2:
                                emit_a2a(0, 0)   # b0 qq0+qq1 staged
                        if jt % 3 == 0 and jt <= 9:
                            nb1 = 4 * (qq - 1) + jt // 3
                            if nb1 < 10:
                                emit_qkv_tile(1, nb1)
                set_carry(0, qq, avp, av, prev_et)
                if qq == 1:
                    nc.sync.dma_start(
                        wprojT.rearrange("p (dt f) -> p dt f", dt=CT),
                        wproj_d.ap().rearrange("(dt p) f -> p dt f", p=P))
            for qq in range(N // QB):
                avp = psV.tile([65, HPC * QB], f32, tag="av", name="avp")
                av = [avp[:, QB * h:QB * (h + 1)] for h in range(HPC)]
                prev_et = None
                for jt in range(TTH):
                    et = emit_scores_exp(1, qq, jt)
                    if jt == 0:
                        flush_carry()
                    else:
                        emit_av(1, qq, jt - 1, av, prev_et)
                    prev_et = et
                    if jt == 2:
                        flush_pend_step(1)
                    elif jt == 4:
                        flush_pend()
                        if qq == 0:
                            emit_a2a(0, 1)   # b0 qq2+qq3 staged
                        elif qq == 2:
                            emit_a2a(1, 0)   # b1 qq0+qq1 staged

                    if qq == 0 and jt % 2 == 1 and jt <= 11:
                        # b1 tiles 10-15: this phase is exp-bound, the PE
                        # has slack; tile 10+j/2 lands >=2 jts before use
                        emit_qkv_tile(1, 10 + jt // 2)
                if qq < 3:
                    set_carry(1, qq, avp, av, prev_et)
                else:
                    final = (avp, av, prev_et)
            # final q-block: everything below is on the tail critical path
            # (last exp -> A2A trigger), so skip the avf PSUM->SBUF copy
            # (psV is never reused) and split the staging across two queues.
            emit_av(1, 3, TTH - 1, final[1], final[2])
            avp = final[0]
            for h in range(HPC):
                sums = wp.tile([1, QB], bf16, tag="sums", name="sums")
                nc.vector.tensor_copy(sums, avp[64:65, QB * h:QB * (h + 1)])
                bc = psA.tile([64, QB], f32, tag="mm", name="bc")
                nc.tensor.matmul(bc, ones_col[:, 0:64], sums,
                                 start=True, stop=True)
                rc = wp.tile([64, QB], f32, tag="recip", bufs=2, name="rc")
                nc.vector.reciprocal_approx_fast(rc, bc)
                nc.vector.tensor_tensor(
                    aoT[HD * h:HD * (h + 1), N + QB * 3:N + QB * 4],
                    avp[0:64, QB * h:QB * (h + 1)], rc, mult)
            aos = aoT[:, N + QB * 3:N + QB * 4].rearrange(
                "p (c t) -> p c t", c=4)
            for k in range(2):
                a2i = a2a_in[1][1][(4 + 2 * k) * P:(6 + 2 * k) * P].rearrange(
                    "(c p) t -> p c t", p=P)
                q = nc.sync if k == 0 else nc.scalar
                q.dma_start(a2i, aos[:, 2 * k:2 * (k + 1)])
            emit_a2a(1, 1)
            # all projection runs in the tail: the attention phase is
            # PE-bound (interleaving proj there costs span 1:1) while the
            # final A2A leaves the PE idle -- overlap it with the three
            # already-delivered halves, then the last one.
            for (pb, ph) in [(0, 0), (0, 1), (1, 0)]:
                for piece in proj_pieces(pb, ph):
                    piece()
            # keep the PE-HAM warm through the final A2A's ~16us peer wait:
            # otherwise the last piece's matmuls run at the cold 1.2GHz clock.
            # ~32 throwaway matmuls (~7-8us warm) fill the idle window; they
            # have no readers and drain before the A2A lands.
            warm = psV.tile([65, HPC * QB], f32, tag="av", name="warm")
            for i in range(32):
                nc.tensor.matmul(warm[:, 0:512], v_sb[:, 0:65], qT[:, 0:512],
                                 start=True, stop=True)
            for piece in proj_pieces(1, 1, split=True):
                piece()

    nc.compile()
    return nc


def _get_nc():
    if "nc" not in _CACHE:
        _CACHE["nc"] = _build_nc()
    return _CACHE["nc"]


def make_in_maps(x, Wqkv, Wproj, bproj, sin, cos):
    """Shard full (f32) inputs into per-core in_maps (pre-cast + pre-transposed)."""
    import ml_dtypes
    bf16 = ml_dtypes.bfloat16
    xT = np.ascontiguousarray(
        np.asarray(x, np.float32).reshape(T, DIM).astype(bf16).T)
    Wqkv = np.asarray(Wqkv, np.float32).astype(bf16)
    WprojT = np.ascontiguousarray(np.asarray(Wproj, np.float32).astype(bf16).T)
    bproj = np.asarray(bproj, np.float32).reshape(1, DIM)
    sin = np.asarray(sin, np.float32)
    cos = np.asarray(cos, np.float32)
    # [n, d] -> [token-in-tile, tile*d] bf16; sneg = rotate-half sign pattern
    sneg = np.concatenate([-sin[:, :HD // 2], sin[:, HD // 2:]], axis=1)
    cosr = np.ascontiguousarray(
        cos.reshape(16, P, HD).transpose(1, 0, 2).reshape(P, 16 * HD)
    ).astype(bf16)
    snegr = np.ascontiguousarray(
        sneg.reshape(16, P, HD).transpose(1, 0, 2).reshape(P, 16 * HD)
    ).astype(bf16)
    ident = np.eye(P, dtype=bf16)
    in_maps = []
    for c in range(NUM_CORES):
        r = P * c
        wq = Wqkv[r:r + P]
        wk = Wqkv[DIM + r:DIM + r + P]
        wv = Wqkv[2 * DIM + r:2 * DIM + r + P]
        in_maps.append({
            "x": xT,
            "wqkv": np.ascontiguousarray(np.concatenate([wq, wk, wv], 0).T),
            "wproj": WprojT,
            "bproj": bproj,
            "cosr": cosr,
            "snegr": snegr,
            "ident": ident,
        })
    return in_maps


def reassemble(outs):
    """outs[c] = [512, 1024] f32, rows [b0h0|b0h1|b1h0|b1h1] x 128 t."""
    out = np.empty((B, N, DIM), np.float32)
    for c in range(NUM_CORES):
        o = outs[c]
        for b in range(B):
            for h in range(2):
                t0 = N // 2 * h + P * c
                out[b, t0:t0 + P] = o[2 * P * b + P * h:2 * P * b + P * (h + 1)]
    return out


def kernel(x, Wqkv, Wproj, bproj, sin, cos):
    from concourse.bass_utils import run_bass_kernel_spmd

    nc = _get_nc()
    in_maps = make_in_maps(x, Wqkv, Wproj, bproj, sin, cos)
    trace = bool(int(os.environ.get("KERNEL_TRACE", "0")))
    res = run_bass_kernel_spmd(nc, in_maps, core_ids=list(range(NUM_CORES)),
                               trace=trace)
    _CACHE["last_result"] = res
    return reassemble([res.results[c]["out"] for c in range(NUM_CORES)])



# revision 66
# speedup vs baseline: 1.0249x; 1.0162x over previous
"""Distributed multi-head attention kernel for one TRN2 chip (8 NeuronCores).

Problem: b=2, n=2048, dim=1024, heads=16, hd=64.
  qkv = x @ Wqkv.T  (qkv-major split) -> RoPE(q,k) -> softmax(q k^T/8) v
  -> merge heads -> @ Wproj.T + bproj

Sharding: each core owns 2 heads (of 16) for BOTH batches. QKV projection,
RoPE and attention are fully head-local. Four 8-way AllToAlls (one per
batch x token-half, 128-token slices per core) redistribute attention
outputs head-major -> token-major as soon as each half's two q-blocks are
done; the first three hide under attention. ALL projection is deferred to
the tail: the attention phase is PE-bound while the final ~20us A2A
(8-rank ncfw latency is ~16us fixed) leaves the PE idle, so the three
already-delivered halves project inside that window. Core c outputs
tokens {1024h+128c : +128} for h in {0,1} of each batch; host reassembles.

Per-core inputs (see make_in_maps) are pre-transposed/pre-cast on the host so
no DMA-xbar transposes are needed (Tile serializes those globally):
  x        [1024, 4096] bf16  x^T: channels x flat tokens
  wqkv     [1024, 384]  bf16  (q|k|v rows for my heads)^T
  wproj    [1024, 1024] bf16  Wproj^T: [d', f]
  bproj    [1, 1024]    f32
  sin/cos  [2048, 64]   f32
  ident    [128, 128]   bf16  identity for PE transposes
  out      [512, 1024]  f32   rows = [b0h0, b0h1, b1h0, b1h1] x 128 tokens

All matmuls bf16 (PSUM accumulates f32). scoresT layout [k_j, q_i] (k
stationary, both heads row-packed across the 128 partitions) so softmax needs
no transposes: denominators come from a ones-column appended to v. exp on
ScalarE with fused 1/8 scale; no max subtraction (scores std ~2). The QKV
chain for each batch is software-pipelined into that batch's first
attention q-block (lag 4 tiles) so PE/ACT streams interleave; in qq0 the
qkc PSUM->SBUF copy rides the half-idle exp stream on ScalarE while the
rest stay on DVE, so neither engine gates the pipeline. b1's last 6 QKV
tiles emit inside b1-qq0, whose exp-bound phase has PE slack. x streams
in 512-token-block descriptors in consumption order; startup-critical
weight/sincos/x descriptors are split across the gpsimd/scalar/sync
queues (descriptor GEN is ~0.6us each and serializes per queue).
"""

import os
import numpy as np

NUM_CORES = 8
B, N, DIM, NH, HD = 2, 2048, 1024, 16, 64
T = B * N                 # 4096 flat tokens
HPC = NH // NUM_CORES     # 2 heads per core
P = 128
CT = DIM // P             # 8 channel tiles
SL = N // NUM_CORES       # 256 output tokens per core per batch
QW = HPC * HD             # 128
FQKV = 3 * QW             # 384
QB = 512                  # attention q-block width
TTH = N // P              # 16 token tiles per batch

_CACHE = {}


def _build_nc():
    from concourse import bacc, mybir, tile

    f32 = mybir.dt.float32
    bf16 = mybir.dt.bfloat16
    Exp = mybir.ActivationFunctionType.Exp
    mult = mybir.AluOpType.mult
    add = mybir.AluOpType.add

    nc = bacc.Bacc("TRN2", target_bir_lowering=False, debug=False,
                   num_devices=NUM_CORES)

    x_d = nc.dram_tensor("x", [DIM, T], bf16, kind="ExternalInput")
    wqkv_d = nc.dram_tensor("wqkv", [DIM, FQKV], bf16, kind="ExternalInput")
    wproj_d = nc.dram_tensor("wproj", [DIM, DIM], bf16, kind="ExternalInput")
    bproj_d = nc.dram_tensor("bproj", [1, DIM], f32, kind="ExternalInput")
    # cos and the rotate-half-negated sin, pre-rearranged on host to
    # [token-in-tile, tile, d] bf16 -- kills the f32 load + DVE negate prep
    # that used to gate the first RoPE by ~6us
    cosr_d = nc.dram_tensor("cosr", [P, 16 * HD], bf16, kind="ExternalInput")
    snegr_d = nc.dram_tensor("snegr", [P, 16 * HD], bf16, kind="ExternalInput")
    ident_d = nc.dram_tensor("ident", [P, P], bf16, kind="ExternalInput")
    out_d = nc.dram_tensor("out", [2 * SL, DIM], f32, kind="ExternalOutput")
    a2a_in = [[nc.dram_tensor(f"a2a_in{b}_{h}", [NUM_CORES * P, P], bf16)
               for h in range(2)] for b in range(B)]
    a2a_out = [[nc.dram_tensor(f"a2a_out{b}_{h}", [NUM_CORES * P, P], bf16)
                for h in range(2)] for b in range(B)]

    with tile.TileContext(nc) as tc:
        with (
            tc.tile_pool(name="persist", bufs=1) as pers,
            tc.tile_pool(name="work", bufs=4) as wp,
            tc.tile_pool(name="expp", bufs=6) as ep,
            tc.tile_pool(name="psA", bufs=2, space="PSUM") as psA,   # qkv/bc/proj/tp
            tc.tile_pool(name="psS", bufs=2, space="PSUM") as psS,   # scores
            tc.tile_pool(name="psV", bufs=1, space="PSUM") as psV,   # av accum
        ):
            # ---------------- persistent SBUF ----------------
            wqkvT = pers.tile([P, CT * FQKV], bf16)     # ct-block: [128c, 384f]
            wprojT = pers.tile([P, CT * DIM], bf16)     # dt-block: [128d', 1024f]
            xT = pers.tile([P, CT * T], bf16)           # ct-block: [128c, 4096t]
            qT = pers.tile([P, T], bf16)                # [d(2 heads), flat t]
            kT = pers.tile([P, T], bf16)
            v_sb = pers.tile([P, HPC * (T // P) * 65], bf16)
            aoT = pers.tile([P, T], bf16)               # [d', flat t]
            aoTr = pers.tile([P, B * NUM_CORES * SL], bf16)  # per b: [d'chnk, 256t]
            cos4 = pers.tile([P, 16 * 4 * HD], bf16)
            sneg4 = pers.tile([P, 16 * 4 * HD], bf16)
            ones_col = pers.tile([1, P], bf16)
            bias_bf = pers.tile([1, DIM], bf16)
            ident = pers.tile([P, P], bf16)

            nc.vector.memset(ones_col, 1.0)
            vv_ones = v_sb.rearrange("p (h t e) -> p h t e", h=HPC, t=T // P)
            nc.vector.memset(vv_ones[:, :, :, 64:65], 1.0)

            # ---------------- prep loads (no xbar transposes) ----------------
            # gpsimd queue: sincos (gate the first RoPE), wqkv (gates the
            # first matmul), ident, bias; sync queue: xT, first token-quarter
            # in half-column passes so tile-0's QKV matmuls start early
            cos_f = wp.tile([P, 16 * HD], bf16, tag="scload2", bufs=1)
            sneg_f = wp.tile([P, 16 * HD], bf16, tag="scload3", bufs=1)
            # descriptor GEN is ~0.6us per dma_start per queue and a single
            # descriptor's transfer runs ~20-60GB/s; split the startup-critical
            # loads across gpsimd + scalar (both free early) and put wqkv ct0
            # (which gates the first matmul) first, halved across 2 engines
            nc.gpsimd.dma_start(wqkvT[:, 0:FQKV // 2], wqkv_d[0:P, 0:FQKV // 2])
            nc.gpsimd.dma_start(wqkvT[:, FQKV // 2:FQKV],
                                wqkv_d[0:P, FQKV // 2:FQKV])
            nc.gpsimd.dma_start(cos_f, cosr_d.ap())
            nc.gpsimd.dma_start(sneg_f, snegr_d.ap())
            for ct in range(1, CT):
                q = nc.gpsimd if ct < 4 else nc.scalar
                q.dma_start(wqkvT[:, FQKV * ct:FQKV * (ct + 1)],
                            wqkv_d[P * ct:P * (ct + 1), :])
            nc.gpsimd.dma_start(ident, ident_d.ap())
            for half in range(2):
                for ct in range(CT):
                    q = nc.scalar if (half == 0 and ct >= 4) else nc.sync
                    q.dma_start(
                        xT[:, T * ct + 512 * half:T * ct + 512 * (half + 1)],
                        x_d[P * ct:P * (ct + 1), 512 * half:512 * (half + 1)])
            xTv = xT.rearrange("p (ct t) -> p ct t", ct=CT)
            xdv = x_d.ap().rearrange("(ct p) t -> p ct t", p=P)
            for blk in range(2, T // 512):   # b0 blocks 2-3, then b1 blocks
                nc.sync.dma_start(xTv[:, :, 512 * blk:512 * (blk + 1)],
                                  xdv[:, :, 512 * blk:512 * (blk + 1)])

            bt = wp.tile([1, DIM], f32, tag="bload", bufs=1)
            nc.gpsimd.dma_start(bt, bproj_d[:, :])
            nc.vector.tensor_copy(bias_bf, bt)
            c4 = cos4.rearrange("p (pt c d) -> p pt c d", pt=16, c=4)
            n4 = sneg4.rearrange("p (pt c d) -> p pt c d", pt=16, c=4)
            cf = cos_f.rearrange("p (pt d) -> p pt d", pt=16)
            nf = sneg_f.rearrange("p (pt d) -> p pt d", pt=16)
            for c in range(4):
                nc.vector.tensor_copy(c4[:, :, c, :], cf)
                nc.vector.tensor_copy(n4[:, :, c, :], nf)

            def emit_qkv_tile(b, tt, act_copies=False):
                """QKV matmul + RoPE + PE transposes for one 128-token tile.

                act_copies: route PSUM->SBUF copies to ScalarE (only safe in
                windows where the exp stream has slack, i.e. b0's first
                q-block)."""
                # act_copies: balanced split -- qkc + one tp copy ride the
                # half-idle qq0 exp stream on ScalarE, the rest stay on DVE
                cp = nc.scalar.copy if act_copies else nc.vector.tensor_copy
                ftt = TTH * b + tt
                qkvp = psA.tile([P, 512], f32, tag="mm", name="qkvp")
                for ct in range(CT):
                    base = T * ct + N * b
                    nc.tensor.matmul(
                        qkvp[:, 0:FQKV],
                        xT[:, base + P * tt:base + P * (tt + 1)],
                        wqkvT[:, FQKV * ct:FQKV * (ct + 1)],
                        start=(ct == 0), stop=(ct == CT - 1))
                qkc = wp.tile([P, 2 * QW], bf16, tag="qkc")
                cp(qkc, qkvp[:, 0:2 * QW])
                pt = tt % 16
                qk3 = qkc.rearrange("p (c d) -> p c d", c=4)
                t1 = wp.tile([P, 2 * QW], bf16, tag="t1")
                t13 = t1.rearrange("p (c d) -> p c d", c=4)
                nc.vector.tensor_tensor(t13[:, :, 0:32], qk3[:, :, 32:64],
                                        n4[:, pt, :, 0:32], mult)
                nc.vector.tensor_tensor(t13[:, :, 32:64], qk3[:, :, 0:32],
                                        n4[:, pt, :, 32:64], mult)
                qkcos = wp.tile([P, 2 * QW], bf16, tag="qkcos")
                nc.vector.tensor_tensor(
                    qkcos, qkc, cos4[:, 4 * HD * pt:4 * HD * (pt + 1)], mult)
                qrope = wp.tile([P, QW], bf16, tag="qrope")
                krope = wp.tile([P, QW], bf16, tag="krope")
                nc.vector.tensor_tensor(qrope, qkcos[:, 0:QW], t1[:, 0:QW], add)
                nc.vector.tensor_tensor(krope, qkcos[:, QW:2 * QW],
                                        t1[:, QW:2 * QW], add)
                # in qq0 the scores slots are mostly idle; borrowing them
                # for the transpose tile lets qkvp double-buffer on psA
                tpool, ttag = (psS, "scores") if act_copies else (psA, "mm")
                tp = tpool.tile([P, 2 * P], bf16, tag=ttag, name="tp")
                nc.tensor.transpose(tp[:, 0:P], qrope, ident)
                nc.tensor.transpose(tp[:, P:2 * P], krope, ident)
                nc.vector.tensor_copy(qT[:, P * ftt:P * (ftt + 1)], tp[:, 0:P])
                nc.vector.tensor_copy(kT[:, P * ftt:P * (ftt + 1)],
                                      tp[:, P:2 * P])
                vv = v_sb.rearrange("p (h t e) -> p h t e", h=HPC, t=T // P)
                nc.vector.tensor_copy(
                    vv[:, :, ftt, 0:HD],
                    qkvp[:, 2 * QW:3 * QW].rearrange("p (h d) -> p h d", h=HPC))

            def emit_scores_exp(b, qq, jt):
                """Scores + exp for one (q-block, j-tile); returns the exp tile."""
                ftt = TTH * b + jt
                sp = psS.tile([P, HPC * QB], f32, tag="scores", name="sp")
                for h in range(HPC):
                    nc.tensor.matmul(
                        sp[:, QB * h:QB * (h + 1)],
                        kT[HD * h:HD * (h + 1), P * ftt:P * (ftt + 1)],
                        qT[HD * h:HD * (h + 1),
                           N * b + QB * qq:N * b + QB * (qq + 1)],
                        start=True, stop=True)
                et = ep.tile([P, HPC * QB], bf16, tag="expT", name="et")
                nc.scalar.activation(et, sp, Exp, scale=float(HD) ** -0.5)
                return et

            def emit_av(b, qq, jt, av, et):
                ftt = TTH * b + jt
                for h in range(HPC):
                    blk = (h * (T // P) + ftt) * 65
                    nc.tensor.matmul(av[h], v_sb[:, blk:blk + 65],
                                     et[:, QB * h:QB * (h + 1)],
                                     start=(jt == 0), stop=(jt == TTH - 1))

            def emit_avf(avp):
                avf = wp.tile([65, HPC * QB], f32, tag="avf", bufs=2, name="avf")
                nc.vector.tensor_copy(avf, avp)
                return avf

            def emit_norm_head(b, qq, avf, h):
                """Denominator broadcast + reciprocal + normalize, one head."""
                sums = wp.tile([1, QB], bf16, tag="sums", name="sums")
                nc.vector.tensor_copy(sums, avf[64:65, QB * h:QB * (h + 1)])
                bc = psA.tile([64, QB], f32, tag="mm", name="bc")
                nc.tensor.matmul(bc, ones_col[:, 0:64], sums,
                                 start=True, stop=True)
                rc = wp.tile([64, QB], f32, tag="recip", bufs=2, name="rc")
                nc.vector.reciprocal_approx_fast(rc, bc)
                nc.vector.tensor_tensor(
                    aoT[HD * h:HD * (h + 1),
                        N * b + QB * qq:N * b + QB * (qq + 1)],
                    avf[0:64, QB * h:QB * (h + 1)], rc, mult)

            def emit_stage(b, qq):
                """A2A staging of this q-block's two 256-token chunks."""
                # stage sub-chunks 4*(qq%2)..+3 of a2a_in[b][qq//2]
                r0 = 4 * P * (qq % 2)
                a2i = a2a_in[b][qq // 2][r0:r0 + 4 * P].rearrange(
                    "(c p) t -> p c t", p=P)
                nc.sync.dma_start(
                    a2i, aoT[:, N * b + QB * qq:N * b + QB * (qq + 1)].rearrange(
                        "p (c t) -> p c t", c=4))

            def emit_a2a(b, h):
                nc.gpsimd.collective_compute(
                    "AllToAll", mybir.AluOpType.bypass,
                    replica_groups=[list(range(NUM_CORES))],
                    ins=[a2a_in[b][h].ap().opt()],
                    outs=[a2a_out[b][h].ap().opt()])

            def proj_pieces(b, h):
                """Fetch A2A(b, h) result and project my 128-token slice.
                Returns emission pieces (fetch, fb0, fb1) for interleaving."""
                rbase = NUM_CORES * P * (2 * b + h)

                def fetch():
                    a2o = a2a_out[b][h].ap().rearrange("(c p) t -> p c t", p=P)
                    nc.sync.dma_start(
                        aoTr[:, rbase:rbase + NUM_CORES * P].rearrange(
                            "p (c t) -> p c t", c=NUM_CORES), a2o)

                def fb_piece(fb):
                    def run():
                        pp = psA.tile([P, 512], f32, tag="mm", name="proj")
                        for dt in range(CT):
                            lo = rbase + P * dt
                            nc.tensor.matmul(
                                pp, aoTr[:, lo:lo + P],
                                wprojT[:, DIM * dt + 512 * fb:DIM * dt + 512 * (fb + 1)],
                                start=(dt == 0), stop=False)
                        nc.tensor.matmul(pp, ones_col[:, 0:P],
                                         bias_bf[:, 512 * fb:512 * (fb + 1)],
                                         start=False, stop=True)
                        ob = wp.tile([P, 512], f32, tag="ob", bufs=2, name="ob")
                        nc.scalar.copy(ob, pp)
                        orow = 2 * P * b + P * h
                        nc.sync.dma_start(
                            out_d[orow:orow + P, 512 * fb:512 * (fb + 1)], ob)
                    return run
                return [fetch, fb_piece(0), fb_piece(1)]



            # ---------------- main schedule ----------------
            # b0: qq0 software-pipelines b0's QKV chain (ScalarE-assisted
            # copies — exp has slack there); qq1-3 interleave b1's QKV tiles
            # (DVE copies). AV matmuls trail scores/exp by one iteration and
            # the last AV + avf copy of each q-block is carried into the next
            # block's first iteration, so the boundary never stalls the exp
            # stream. Batch-0's A2A + projection hide under b1's attention.
            LAG = 4
            state = {"pend": None, "carry": None}

            def flush_carry():
                if state["carry"] is not None:
                    state["carry"]()
                    state["carry"] = None

            def set_carry(b, qq, avp, av, et):
                def fn():
                    emit_av(b, qq, TTH - 1, av, et)
                    state["pend"] = (b, qq, emit_avf(avp), 0)
                state["carry"] = fn

            def flush_pend_step(upto):
                """Emit the pending norm up to head `upto`; upto==2 also
                stages. Spreading the two heads' broadcast matmuls across
                jt slots avoids bunching PE work at one qq-boundary slot."""
                if state["pend"] is None:
                    return
                b, qq, avf, nh = state["pend"]
                while nh < min(upto, HPC):
                    emit_norm_head(b, qq, avf, nh)
                    nh += 1
                if upto >= 2:
                    emit_stage(b, qq)
                    state["pend"] = None
                else:
                    state["pend"] = (b, qq, avf, nh)

            def flush_pend():
                flush_pend_step(2)

            for qq in range(N // QB):
                avp = psV.tile([65, HPC * QB], f32, tag="av", name="avp")
                av = [avp[:, QB * h:QB * (h + 1)] for h in range(HPC)]
                prev_et = None
                if qq == 0:
                    for step in range(TTH + LAG):
                        if step < TTH:
                            emit_qkv_tile(0, step, act_copies=True)
                        if step >= LAG:
                            jt = step - LAG
                            et = emit_scores_exp(0, qq, jt)
                            if jt >= 1:
                                emit_av(0, qq, jt - 1, av, prev_et)
                            prev_et = et
                else:
                    for jt in range(TTH):
                        et = emit_scores_exp(0, qq, jt)
                        if jt == 0:
                            flush_carry()
                        else:
                            emit_av(0, qq, jt - 1, av, prev_et)
                        prev_et = et
                        if jt == 2:
                            flush_pend_step(1)
                        elif jt == 4:
                            flush_pend()
                            if qq == 2:
                                emit_a2a(0, 0)   # b0 qq0+qq1 staged
                        if jt % 3 == 0 and jt <= 9:
                            nb1 = 4 * (qq - 1) + jt // 3
                            if nb1 < 10:
                                emit_qkv_tile(1, nb1)
                set_carry(0, qq, avp, av, prev_et)
                if qq == 1:
                    nc.sync.dma_start(
                        wprojT.rearrange("p (dt f) -> p dt f", dt=CT),
                        wproj_d.ap().rearrange("(dt p) f -> p dt f", p=P))
            for qq in range(N // QB):
                avp = psV.tile([65, HPC * QB], f32, tag="av", name="avp")
                av = [avp[:, QB * h:QB * (h + 1)] for h in range(HPC)]
                prev_et = None
                for jt in range(TTH):
                    et = emit_scores_exp(1, qq, jt)
                    if jt == 0:
                        flush_carry()
                    else:
                        emit_av(1, qq, jt - 1, av, prev_et)
                    prev_et = et
                    if jt == 2:
                        flush_pend_step(1)
                    elif jt == 4:
                        flush_pend()
                        if qq == 0:
                            emit_a2a(0, 1)   # b0 qq2+qq3 staged
                        elif qq == 2:
                            emit_a2a(1, 0)   # b1 qq0+qq1 staged

                    if qq == 0 and jt % 2 == 1 and jt <= 11:
                        # b1 tiles 10-15: this phase is exp-bound, the PE
                        # has slack; tile 10+j/2 lands >=2 jts before use
                        emit_qkv_tile(1, 10 + jt // 2)
                set_carry(1, qq, avp, av, prev_et)
            flush_carry()
            flush_pend()
            emit_a2a(1, 1)
            # all projection runs in the tail: the attention phase is
            # PE-bound (interleaving proj there costs span 1:1) while the
            # final A2A leaves the PE idle -- overlap it with the three
            # already-delivered halves, then the last one.
            for (pb, ph) in [(0, 0), (0, 1), (1, 0), (1, 1)]:
                for piece in proj_pieces(pb, ph):
                    piece()

    nc.compile()
    return nc


def _get_nc():
    if "nc" not in _CACHE:
        _CACHE["nc"] = _build_nc()
    return _CACHE["nc"]


def make_in_maps(x, Wqkv, Wproj, bproj, sin, cos):
    """Shard full (f32) inputs into per-core in_maps (pre-cast + pre-transposed)."""
    import ml_dtypes
    bf16 = ml_dtypes.bfloat16
    xT = np.ascontiguousarray(
        np.asarray(x, np.float32).reshape(T, DIM).astype(bf16).T)
    Wqkv = np.asarray(Wqkv, np.float32).astype(bf16)
    WprojT = np.ascontiguousarray(np.asarray(Wproj, np.float32).astype(bf16).T)
    bproj = np.asarray(bproj, np.float32).reshape(1, DIM)
    sin = np.asarray(sin, np.float32)
    cos = np.asarray(cos, np.float32)
    # [n, d] -> [token-in-tile, tile*d] bf16; sneg = rotate-half sign pattern
    sneg = np.concatenate([-sin[:, :HD // 2], sin[:, HD // 2:]], axis=1)
    cosr = np.ascontiguousarray(
        cos.reshape(16, P, HD).transpose(1, 0, 2).reshape(P, 16 * HD)
    ).astype(bf16)
    snegr = np.ascontiguousarray(
        sneg.reshape(16, P, HD).transpose(1, 0, 2).reshape(P, 16 * HD)
    ).astype(bf16)
    ident = np.eye(P, dtype=bf16)
    in_maps = []
    for c in range(NUM_CORES):
        r = P * c
        wq = Wqkv[r:r + P]
        wk = Wqkv[DIM + r:DIM + r + P]
        wv = Wqkv[2 * DIM + r:2 * DIM + r + P]
        in_maps.append({
            "x": xT,
            "wqkv": np.ascontiguousarray(np.concatenate([wq, wk, wv], 0).T),
            "wproj": WprojT,
            "bproj": bproj,
            "cosr": cosr,
            "snegr": snegr,
            "ident": ident,
        })
    return in_maps


def reassemble(outs):
    """outs[c] = [512, 1024] f32, rows [b0h0|b0h1|b1h0|b1h1] x 128 t."""
    out = np.empty((B, N, DIM), np.float32)
    for c in range(NUM_CORES):
        o = outs[c]
        for b in range(B):
            for h in range(2):
                t0 = N // 2 * h + P * c
                out[b, t0:t0 + P] = o[2 * P * b + P * h:2 * P * b + P * (h + 1)]
    return out


def kernel(x, Wqkv, Wproj, bproj, sin, cos):
    from concourse.bass_utils import run_bass_kernel_spmd

    nc = _get_nc()
    in_maps = make_in_maps(x, Wqkv, Wproj, bproj, sin, cos)
    trace = bool(int(os.environ.get("KERNEL_TRACE", "0")))
    res = run_bass_kernel_spmd(nc, in_maps, core_ids=list(range(NUM_CORES)),
                               trace=trace)
    _CACHE["last_result"] = res
    return reassemble([res.results[c]["out"] for c in range(NUM_CORES)])

